# revision 2
# baseline (speedup 1.0000x reference)
"""fp8-DoubleRow Bass/Tile kernel for the MoE block (v7).

Sharding: 8 cores = 4 batches x 2 H-halves; each core computes a full
[96, 128, 256] output slab from a host-padded [96, 140, 268] bf16 slab
plus bf16 per-pixel LayerNorm scale/shift planes (host-computed,
partition-broadcast onto SBUF by DMA).

All three experts are folded into proj as 96x96 tap matmuls on xn:
  e1/e2 taps:   proj . diag((1+prompt) * w_k * dw_tap)
  e0 taps:      proj . diag(...) . pw      (pointwise conv pre-composed)
Same-offset taps merge, giving 41 unique taps -> 20 fp8 DoubleRow pairs
(two taps per matmul; rhs 3D [96, 2, 256] with even pair step — HW
constraint) + 1 single fp8 matmul per output row. ffn2 (K=192) is one
DoubleRow matmul on fp8 gelu outputs. fp8 weight scales are powers of
two, descaled via ACT scale= / DVE STT scalar.

Per-chunk tails (STT shortcut-add, ffn1, gelu, ffn2, out) run as a
two-stage software pipeline one/two tap-groups behind, across block
boundaries, so the PE never waits on freshly produced DVE/ACT data.
"""
import os
import sys

os.environ.setdefault("MYCRO_LOCAL_CACHE", "1")

import numpy as np

for _p in ("/opt/trn_rl_repo",):
    if _p not in sys.path:
        sys.path.append(_p)

import concourse.bass as bass  # noqa: E402
import concourse.bacc as bacc  # noqa: E402
import concourse.tile as tile  # noqa: E402
from concourse import mybir  # noqa: E402
from concourse.bass_utils import run_bass_kernel_spmd  # noqa: E402

F32 = mybir.dt.float32
BF16 = mybir.dt.bfloat16
FP8 = mybir.dt.float8e4
NPBF = mybir.dt.np(BF16)
NP8 = mybir.dt.np(FP8)
OP = mybir.AluOpType
AF = mybir.ActivationFunctionType
DRMODE = mybir.MatmulPerfMode.DoubleRow

DIM = 96
B, H, W = 4, 256, 256
HALO = 6
BH = 16                      # output rows per block
NBLK = (H // 2) // BH        # 8
WP = W + 2 * HALO            # 268
ROWS = BH + 2 * HALO         # 28
NPAD = ROWS * WP             # 7504
CH = 512
NCHUNK = (NPAD + CH - 1) // CH   # 15
NXC = BH * W // CH           # 8 x1/ffn chunks per block (2 rows each)
EPS = 1e-6
TAP_GROUP = 2

TAPS_E1 = [(di, dj) for di in (-2, 0, 2) for dj in (-2, 0, 2)]
TAPS_E2 = [(di, dj) for di in (-6, -3, 0, 3, 6) for dj in (-6, -3, 0, 3, 6)]
TAPS_E0 = [(di, dj) for di in (-1, 0, 1) for dj in (-1, 0, 1)]

# unique merged tap offsets, deterministic order
ALL_OFFS = sorted(set(TAPS_E1) | set(TAPS_E2) | set(TAPS_E0))
_EV = [t for t in ALL_OFFS if t[1] % 2 == 0]
_OD = [t for t in ALL_OFFS if t[1] % 2 != 0]
PAIRS = [(_EV[i], _EV[i + 1]) for i in range(0, len(_EV) - 1, 2)] + \
        [(_OD[i], _OD[i + 1]) for i in range(0, len(_OD) - 1, 2)]
SINGLES = ([_EV[-1]] if len(_EV) % 2 else []) + \
          ([_OD[-1]] if len(_OD) % 2 else [])
assert len(SINGLES) == 1 and len(PAIRS) == 20
NPAIR = len(PAIRS)

_CACHE = {}


def build_nc(reps=1):
    key = ("nc", reps)
    if key in _CACHE:
        return _CACHE[key]
    nc = bacc.Bacc("TRN2", target_bir_lowering=False, debug=False)

    xp_d = nc.dram_tensor("xp", [DIM, H // 2 + 2 * HALO, WP], BF16,
                          kind="ExternalInput")
    ab_d = nc.dram_tensor("ab", [2, H // 2 + 2 * HALO, WP], BF16,
                          kind="ExternalInput")
    wtp_d = nc.dram_tensor("wtp", [NPAIR, DIM, 2, DIM], FP8,
                           kind="ExternalInput")
    wts_d = nc.dram_tensor("wts", [DIM, DIM], FP8, kind="ExternalInput")
    wf1_d = nc.dram_tensor("wf1", [DIM, 2 * DIM], BF16, kind="ExternalInput")
    wf2_d = nc.dram_tensor("wf2", [DIM, 2, DIM], FP8, kind="ExternalInput")
    bias_d = nc.dram_tensor("bias", [DIM, 8], F32, kind="ExternalInput")
    y_d = nc.dram_tensor("y", [DIM, H // 2, W], F32, kind="ExternalOutput")

    with tile.TileContext(nc) as tc:
        _emit(nc, tc, xp_d, ab_d, wtp_d, wts_d, wf1_d, wf2_d, bias_d, y_d,
              reps)

    nc.compile()
    _CACHE[key] = nc
    return nc


def _emit(nc, tc, xp_d, ab_d, wtp_d, wts_d, wf1_d, wf2_d, bias_d, y_d,
          reps=1):
    pools = []

    wpool = tc.alloc_tile_pool(name="w", bufs=1)
    xpool = tc.alloc_tile_pool(name="xp", bufs=2)
    xnpool = tc.alloc_tile_pool(name="xn", bufs=2)
    tmppool = tc.alloc_tile_pool(name="tmp", bufs=2)
    abpool = tc.alloc_tile_pool(name="abp", bufs=2)
    x1pool = tc.alloc_tile_pool(name="x1", bufs=5)
    hbpool = tc.alloc_tile_pool(name="hb", bufs=5)
    opool = tc.alloc_tile_pool(name="o", bufs=4)
    ps_acc = tc.alloc_tile_pool(name="pacc", bufs=2 * TAP_GROUP, space="PSUM")
    ps_h = tc.alloc_tile_pool(name="ph", bufs=1, space="PSUM")
    ps_o = tc.alloc_tile_pool(name="po", bufs=2, space="PSUM")
    pools += [wpool, xpool, xnpool, tmppool, abpool, x1pool, hbpool,
              opool, ps_acc, ps_h, ps_o]

    # ---- weights / constants (loaded once) ----
    wtp_sb = wpool.tile([DIM, NPAIR, 2, DIM], FP8)
    nc.sync.dma_start(wtp_sb[:], wtp_d.ap().rearrange("t c k o -> c t k o"))
    wts_sb = wpool.tile([DIM, DIM], FP8)
    nc.sync.dma_start(wts_sb[:], wts_d[:])
    wf1_sb = wpool.tile([DIM, 2 * DIM], BF16)
    nc.sync.dma_start(wf1_sb[:], wf1_d[:])
    wf2_sb = wpool.tile([DIM, 2, DIM], FP8)
    nc.sync.dma_start(wf2_sb[:], wf2_d[:])
    bias_sb = wpool.tile([DIM, 8], F32)
    nc.sync.dma_start(bias_sb[:], bias_d[:])

    b_f1a = bias_sb[:, 2:3]
    b_f1b = bias_sb[:, 3:4]
    b_f2 = bias_sb[:, 4:5]
    inv_s = bias_sb[:, 5:6]      # 1/s for tap weights
    inv_s2 = bias_sb[:, 7:8]     # 1/s2 for ffn2

    def stage_A(i):
        """DMA (incl. partition-broadcast LN planes) + normalize block i."""
        r0 = BH * i
        xf = xpool.tile([DIM, ROWS, WP], BF16, tag="xf", name="xf")
        nc.sync.dma_start(xf[:], xp_d[:, r0:r0 + ROWS, :])
        xfF = xf.rearrange("p r w -> p (r w)")
        ab_bc = abpool.tile([DIM, 2, ROWS, WP], BF16, tag="ab", name="ab_bc")
        for pl in range(2):
            src = ab_d[pl:pl + 1, r0:r0 + ROWS, :]
            src = bass.AP(src.tensor, src.offset,
                          [[0, DIM]] + [list(q) for q in src.ap[1:]])
            nc.sync.dma_start(ab_bc[:, pl], src)
        aF = ab_bc[:, 0].rearrange("p r w -> p (r w)")
        bF = ab_bc[:, 1].rearrange("p r w -> p (r w)")
        xn = xnpool.tile([DIM, ROWS, WP], FP8, tag="xn", name="xn")
        xnF = xn.rearrange("p r w -> p (r w)")
        for j in range(NCHUNK):
            c0 = j * CH
            nj = min(CH, NPAD - c0)
            tmp = tmppool.tile([DIM, CH], BF16, tag="t", name="tmp")
            nc.vector.tensor_mul(tmp[:, :nj], xfF[:, c0:c0 + nj],
                                 aF[:, c0:c0 + nj])
            nc.gpsimd.tensor_add(xnF[:, c0:c0 + nj], tmp[:, :nj],
                                 bF[:, c0:c0 + nj])
        return dict(xf=xf, xn=xn)

    # -- two-stage tail pipeline (runs behind the tap matmuls, across
    #    blocks): stage1 = STT shortcut add + ffn1 + gelu; stage2 = ffn2 +
    #    descale + residual + output DMA. k is the global chunk index.
    pend1, pend2 = [], []

    def tail1(k, x1ps, xf, kl):
        x1b = x1pool.tile([DIM, CH], BF16, tag="x1b", name="x1b")
        nc.vector.scalar_tensor_tensor(
            x1b[:], x1ps[:], inv_s,
            xf[:, 6 + 2 * kl:8 + 2 * kl, HALO:W + HALO], OP.mult, OP.add)
        hps = ps_h.tile([DIM, 2 * CH], F32, tag="h", name="hps")
        nc.tensor.matmul(hps[:, :CH], wf1_sb[:, 0:DIM], x1b[:],
                         start=True, stop=True)
        nc.tensor.matmul(hps[:, CH:], wf1_sb[:, DIM:2 * DIM], x1b[:],
                         start=True, stop=True)
        hb = hbpool.tile([DIM, 2, CH], FP8, tag="hb", name="hb")
        nc.scalar.activation(hb[:, 0, :], hps[:, :CH], AF.Gelu, bias=b_f1a)
        nc.scalar.activation(hb[:, 1, :], hps[:, CH:], AF.Gelu, bias=b_f1b)
        return x1b, hb

    def tail2(k, x1b, hb):
        ops_ = ps_o.tile([DIM, CH], F32, tag="o", name="ops_")
        hb_ap = hb.rearrange("p k c -> p (k c)")
        part = list(hb_ap.ap[0])
        rhs_h = bass.AP(hb_ap.tensor, hb_ap.offset, [part, [CH, 2], [1, CH]])
        nc.tensor.matmul(ops_[:], wf2_sb[:], rhs_h, start=True, stop=True,
                         perf_mode=DRMODE)
        ot = opool.tile([DIM, CH], BF16, tag="ot", name="ot")
        nc.scalar.activation(ot[:], ops_[:], AF.Identity, bias=b_f2,
                             scale=inv_s2)
        out_c = opool.tile([DIM, 2, W], F32, tag="out", name="out_c")
        nc.vector.tensor_add(out_c[:], ot[:], x1b[:])
        nc.sync.dma_start(y_d[:, 2 * k:2 * k + 2, :], out_c[:])

    def drain(n1, n2):
        while len(pend1) > n1:
            k, acc, xf, kl = pend1.pop(0)
            x1b, hb = tail1(k, acc, xf, kl)
            pend2.append((k, x1b, hb))
        while len(pend2) > n2:
            tail2(*pend2.pop(0))

    def stage_B(i, st):
        """paired-tap matmul groups for block i (+ pipelined tails)."""
        xf, xn = st["xf"], st["xn"]
        xn_flat = xn.rearrange("p r w -> p (r w)")
        part = list(xn_flat.ap[0])

        def off(row, di, dj):
            return (6 + row + di) * WP + HALO + dj

        for g in range(NXC // TAP_GROUP):
            accs = [ps_acc.tile([DIM, CH], F32, tag="acc", name="x1ps")
                    for _ in range(TAP_GROUP)]
            for p, ((diA, djA), (diB, djB)) in enumerate(PAIRS):
                step = (diB - diA) * WP + (djB - djA)
                for q in range(TAP_GROUP):
                    for h in range(2):
                        row = (TAP_GROUP * g + q) * 2 + h
                        rhs = bass.AP(xn_flat.tensor,
                                      xn_flat.offset + off(row, diA, djA),
                                      [part, [step, 2], [1, W]])
                        nc.tensor.matmul(accs[q][:, h * W:(h + 1) * W],
                                         wtp_sb[:, p, :, :], rhs,
                                         start=(p == 0 and h == 0),
                                         stop=False, perf_mode=DRMODE)
            (diS, djS) = SINGLES[0]
            for q in range(TAP_GROUP):
                for h in range(2):
                    row = (TAP_GROUP * g + q) * 2 + h
                    o = off(row, diS, djS)
                    nc.tensor.matmul(accs[q][:, h * W:(h + 1) * W],
                                     wts_sb[:], xn_flat[:, o:o + W],
                                     start=False, stop=(h == 1))
            drain(0, TAP_GROUP)
            pend1.extend(
                (NXC * i + TAP_GROUP * g + q, accs[q], xf, TAP_GROUP * g + q)
                for q in range(TAP_GROUP))

    from contextlib import ExitStack
    rep_ctx = ExitStack()
    if reps > 1:
        rep_ctx.enter_context(tc.For_i(0, reps, 1))
    st = stage_A(0)
    for i in range(NBLK):
        nxt = stage_A(i + 1) if i + 1 < NBLK else None
        stage_B(i, st)
        st = nxt
    drain(0, 0)

    rep_ctx.close()

    for p in reversed(pools):
        p.release()


# ---------------- host side ----------------

def _p2_scale(maxval, cap=224.0):
    """Largest power of two s with s*maxval <= cap."""
    if maxval <= 0:
        return 1.0
    return 2.0 ** int(np.floor(np.log2(cap / maxval)))


def prep_core(inputs, core):
    b, half = core // 2, core % 2
    x = np.asarray(inputs["x"][b], np.float32)
    xp = np.zeros((DIM, H // 2 + 2 * HALO, WP), np.float32)
    r_lo = half * (H // 2) - HALO
    s_lo, s_hi = max(0, r_lo), min(H, r_lo + H // 2 + 2 * HALO)
    xp[:, s_lo - r_lo:s_hi - r_lo, HALO:W + HALO] = x[:, s_lo:s_hi, :]

    # per-pixel LN stats planes (note: zero-padded pixels give a = 1/sqrt(eps)
    # but b = 0 and x = 0, so xn = 0 in the halo, matching conv zero-pad)
    s1 = xp.sum(axis=0)
    s2 = (xp * xp).sum(axis=0)
    var = s2 / DIM - (s1 / DIM) ** 2
    rsig = 1.0 / np.sqrt(var + EPS)
    ab = np.stack([rsig, -(s1 / DIM) * rsig]).astype(NPBF)

    w0, w1, w2 = [float(v) for v in np.asarray(inputs["scale_weights"][b],
                                               np.float64)]
    s = 1.0 + np.asarray(inputs["prompt"][b], np.float64)
    projW_s = np.asarray(inputs["proj_w"], np.float64) * s[None, :]

    e0k = np.asarray(inputs["e0_dw_w"], np.float64)[:, 0]   # [96,3,3]
    e1k = np.asarray(inputs["e1_dw_w"], np.float64)[:, 0]
    e2k = np.asarray(inputs["e2_dw_w"], np.float64)[:, 0]
    pw = np.asarray(inputs["e0_pw_w"], np.float64)          # [out, in]
    b_pw = np.asarray(inputs["e0_pw_b"], np.float64)

    # merged folded tap matrices, lhsT layout [in(xn ch), out]
    folded = {t: np.zeros((DIM, DIM)) for t in ALL_OFFS}
    for (di, dj) in TAPS_E1:
        col = w1 * e1k[:, di // 2 + 1, dj // 2 + 1]
        folded[(di, dj)] += (projW_s * col[None, :]).T
    for (di, dj) in TAPS_E2:
        col = w2 * e2k[:, di // 3 + 2, dj // 3 + 2]
        folded[(di, dj)] += (projW_s * col[None, :]).T
    for (di, dj) in TAPS_E0:
        col = w0 * e0k[:, di + 1, dj + 1]
        folded[(di, dj)] += ((projW_s * col[None, :]) @ pw).T

    s_tap = _p2_scale(max(np.abs(m).max() for m in folded.values()))
    wtp = np.zeros((NPAIR, DIM, 2, DIM), np.float64)
    for p, (tA, tB) in enumerate(PAIRS):
        wtp[p, :, 0, :] = folded[tA] * s_tap
        wtp[p, :, 1, :] = folded[tB] * s_tap
    wts = folded[SINGLES[0]] * s_tap

    f2T = np.asarray(inputs["ffn2_w"], np.float64).T    # [192, 96]
    s_f2 = _p2_scale(np.abs(f2T).max())
    wf2 = np.stack([f2T[:DIM], f2T[DIM:]], axis=1) * s_f2  # [96, 2, 96]

    # effective proj bias: dw biases through proj, plus the e0 pointwise
    # bias routed through the composed e0 taps
    cb = (w1 * np.asarray(inputs["e1_dw_b"], np.float64)
          + w2 * np.asarray(inputs["e2_dw_b"], np.float64)
          + w0 * np.asarray(inputs["e0_dw_b"], np.float64))
    e0_colsum = w0 * e0k.sum(axis=(1, 2))
    proj_b_eff = (np.asarray(inputs["proj_b"], np.float64) + projW_s @ cb
                  + projW_s @ (e0_colsum * b_pw))

    ln_b = np.asarray(inputs["ln_b"], np.float64)
    assert np.allclose(ln_b, 0.0), "kernel folds ln_b=0; got nonzero ln_b"

    # b_proj folded into ffn1 bias and the output residual constant
    f1w = np.asarray(inputs["ffn1_w"], np.float64)
    f1b_eff = np.asarray(inputs["ffn1_b"], np.float64) + f1w @ proj_b_eff
    bias = np.zeros((DIM, 8), np.float64)
    bias[:, 2] = f1b_eff[:DIM]
    bias[:, 3] = f1b_eff[DIM:]
    bias[:, 4] = np.asarray(inputs["ffn2_b"], np.float64) + proj_b_eff
    bias[:, 5] = 1.0 / s_tap
    bias[:, 7] = 1.0 / s_f2

    clip8 = lambda a: np.clip(a, -224.0, 224.0).astype(NP8)
    return {
        "xp": xp.astype(NPBF),
        "ab": ab,
        "wtp": clip8(wtp),
        "wts": clip8(wts),
        "wf1": np.asarray(inputs["ffn1_w"], np.float64).T.astype(NPBF),
        "wf2": clip8(wf2),
        "bias": bias.astype(np.float32),
    }


def kernel(**inputs):
    nc = build_nc()
    in_maps = [prep_core(inputs, c) for c in range(8)]
    res = run_bass_kernel_spmd(nc, in_maps, list(range(8)))
    out = np.empty((B, DIM, H, W), np.float32)
    for c in range(8):
        b, half = c // 2, c % 2
        out[b, :, half * (H // 2):(half + 1) * (H // 2), :] = res.results[c]["y"]
    return out


# revision 3
# speedup vs baseline: 1.0511x; 1.0511x over previous
"""fp8-DoubleRow Bass/Tile kernel for the MoE block (v7).

Sharding: 8 cores = 4 batches x 2 H-halves; each core computes a full
[96, 128, 256] output slab from a host-padded [96, 140, 268] bf16 slab
plus bf16 per-pixel LayerNorm scale/shift planes (host-computed,
partition-broadcast onto SBUF by DMA).

All three experts are folded into proj as 96x96 tap matmuls on xn:
  e1/e2 taps:   proj . diag((1+prompt) * w_k * dw_tap)
  e0 taps:      proj . diag(...) . pw      (pointwise conv pre-composed)
Same-offset taps merge, giving 41 unique taps -> 20 fp8 DoubleRow pairs
(two taps per matmul; rhs 3D [96, 2, 256] with even pair step — HW
constraint) + 1 single fp8 matmul per output row. ffn2 (K=192) is one
DoubleRow matmul on fp8 gelu outputs. fp8 weight scales are powers of
two, descaled via ACT scale= / DVE STT scalar.

Per-chunk tails (STT shortcut-add, ffn1, gelu, ffn2, out) run as a
two-stage software pipeline one/two tap-groups behind, across block
boundaries, so the PE never waits on freshly produced DVE/ACT data.
"""
import os
import sys

os.environ.setdefault("MYCRO_LOCAL_CACHE", "1")

import numpy as np

for _p in ("/opt/trn_rl_repo",):
    if _p not in sys.path:
        sys.path.append(_p)

import concourse.bass as bass  # noqa: E402
import concourse.bacc as bacc  # noqa: E402
import concourse.tile as tile  # noqa: E402
from concourse import mybir  # noqa: E402
from concourse.bass_utils import run_bass_kernel_spmd  # noqa: E402

F32 = mybir.dt.float32
BF16 = mybir.dt.bfloat16
FP8 = mybir.dt.float8e4
NPBF = mybir.dt.np(BF16)
NP8 = mybir.dt.np(FP8)
OP = mybir.AluOpType
AF = mybir.ActivationFunctionType
DRMODE = mybir.MatmulPerfMode.DoubleRow

DIM = 96
B, H, W = 4, 256, 256
HALO = 6
BH = 16                      # output rows per block
NBLK = (H // 2) // BH        # 8
WP = W + 2 * HALO            # 268
ROWS = BH + 2 * HALO         # 28
NPAD = ROWS * WP             # 7504
CH = 512
NCHUNK = (NPAD + CH - 1) // CH   # 15
NXC = BH * W // CH           # 8 x1/ffn chunks per block (2 rows each)
EPS = 1e-6
TAP_GROUP = 2

TAPS_E1 = [(di, dj) for di in (-2, 0, 2) for dj in (-2, 0, 2)]
TAPS_E2 = [(di, dj) for di in (-6, -3, 0, 3, 6) for dj in (-6, -3, 0, 3, 6)]
TAPS_E0 = [(di, dj) for di in (-1, 0, 1) for dj in (-1, 0, 1)]

# unique merged tap offsets, deterministic order
ALL_OFFS = sorted(set(TAPS_E1) | set(TAPS_E2) | set(TAPS_E0))
_EV = [t for t in ALL_OFFS if t[1] % 2 == 0]
_OD = [t for t in ALL_OFFS if t[1] % 2 != 0]
PAIRS = [(_EV[i], _EV[i + 1]) for i in range(0, len(_EV) - 1, 2)] + \
        [(_OD[i], _OD[i + 1]) for i in range(0, len(_OD) - 1, 2)]
SINGLES = ([_EV[-1]] if len(_EV) % 2 else []) + \
          ([_OD[-1]] if len(_OD) % 2 else [])
assert len(SINGLES) == 1 and len(PAIRS) == 20
NPAIR = len(PAIRS)

_CACHE = {}


def build_nc(reps=1):
    key = ("nc", reps)
    if key in _CACHE:
        return _CACHE[key]
    nc = bacc.Bacc("TRN2", target_bir_lowering=False, debug=False)

    xp_d = nc.dram_tensor("xp", [DIM, H // 2 + 2 * HALO, WP], BF16,
                          kind="ExternalInput")
    ab_d = nc.dram_tensor("ab", [2, H // 2 + 2 * HALO, WP], BF16,
                          kind="ExternalInput")
    wtp_d = nc.dram_tensor("wtp", [NPAIR, DIM, 2, DIM], FP8,
                           kind="ExternalInput")
    wts_d = nc.dram_tensor("wts", [DIM, DIM], FP8, kind="ExternalInput")
    wf1_d = nc.dram_tensor("wf1", [DIM, 2 * DIM], BF16, kind="ExternalInput")
    wf2_d = nc.dram_tensor("wf2", [DIM, 2, DIM], FP8, kind="ExternalInput")
    bias_d = nc.dram_tensor("bias", [DIM, 8], F32, kind="ExternalInput")
    y_d = nc.dram_tensor("y", [DIM, H // 2, W], BF16,
                         kind="ExternalOutput")

    with tile.TileContext(nc) as tc:
        _emit(nc, tc, xp_d, ab_d, wtp_d, wts_d, wf1_d, wf2_d, bias_d, y_d,
              reps)

    nc.compile()
    _CACHE[key] = nc
    return nc


def _emit(nc, tc, xp_d, ab_d, wtp_d, wts_d, wf1_d, wf2_d, bias_d, y_d,
          reps=1):
    pools = []

    wpool = tc.alloc_tile_pool(name="w", bufs=1)
    xpool = tc.alloc_tile_pool(name="xp", bufs=2)
    xnpool = tc.alloc_tile_pool(name="xn", bufs=2)
    tmppool = tc.alloc_tile_pool(name="tmp", bufs=2)
    abpool = tc.alloc_tile_pool(name="abp", bufs=2)
    x1pool = tc.alloc_tile_pool(name="x1", bufs=5)
    hbpool = tc.alloc_tile_pool(name="hb", bufs=5)
    opool = tc.alloc_tile_pool(name="o", bufs=4)
    ps_acc = tc.alloc_tile_pool(name="pacc", bufs=2 * TAP_GROUP, space="PSUM")
    ps_h = tc.alloc_tile_pool(name="ph", bufs=1, space="PSUM")
    ps_o = tc.alloc_tile_pool(name="po", bufs=2, space="PSUM")
    pools += [wpool, xpool, xnpool, tmppool, abpool, x1pool, hbpool,
              opool, ps_acc, ps_h, ps_o]

    # ---- weights / constants (loaded once) ----
    wtp_sb = wpool.tile([DIM, NPAIR, 2, DIM], FP8)
    nc.sync.dma_start(wtp_sb[:], wtp_d.ap().rearrange("t c k o -> c t k o"))
    wts_sb = wpool.tile([DIM, DIM], FP8)
    nc.sync.dma_start(wts_sb[:], wts_d[:])
    wf1_sb = wpool.tile([DIM, 2 * DIM], BF16)
    nc.sync.dma_start(wf1_sb[:], wf1_d[:])
    wf2_sb = wpool.tile([DIM, 2, DIM], FP8)
    nc.sync.dma_start(wf2_sb[:], wf2_d[:])
    bias_sb = wpool.tile([DIM, 8], F32)
    nc.sync.dma_start(bias_sb[:], bias_d[:])

    b_f1a = bias_sb[:, 2:3]
    b_f1b = bias_sb[:, 3:4]
    b_f2 = bias_sb[:, 4:5]
    inv_s = bias_sb[:, 5:6]      # 1/s for tap weights
    inv_s2 = bias_sb[:, 7:8]     # 1/s2 for ffn2

    def stage_A(i):
        """DMA (incl. partition-broadcast LN planes) + normalize block i."""
        r0 = BH * i
        xf = xpool.tile([DIM, ROWS, WP], BF16, tag="xf", name="xf")
        nc.sync.dma_start(xf[:], xp_d[:, r0:r0 + ROWS, :])
        xfF = xf.rearrange("p r w -> p (r w)")
        ab_bc = abpool.tile([DIM, 2, ROWS, WP], BF16, tag="ab", name="ab_bc")
        for pl in range(2):
            src = ab_d[pl:pl + 1, r0:r0 + ROWS, :]
            src = bass.AP(src.tensor, src.offset,
                          [[0, DIM]] + [list(q) for q in src.ap[1:]])
            nc.sync.dma_start(ab_bc[:, pl], src)
        aF = ab_bc[:, 0].rearrange("p r w -> p (r w)")
        bF = ab_bc[:, 1].rearrange("p r w -> p (r w)")
        xn = xnpool.tile([DIM, ROWS, WP], FP8, tag="xn", name="xn")
        xnF = xn.rearrange("p r w -> p (r w)")
        for j in range(NCHUNK):
            c0 = j * CH
            nj = min(CH, NPAD - c0)
            tmp = tmppool.tile([DIM, CH], BF16, tag="t", name="tmp")
            nc.vector.tensor_mul(tmp[:, :nj], xfF[:, c0:c0 + nj],
                                 aF[:, c0:c0 + nj])
            nc.gpsimd.tensor_add(xnF[:, c0:c0 + nj], tmp[:, :nj],
                                 bF[:, c0:c0 + nj])
        return dict(xf=xf, xn=xn)

    # -- two-stage tail pipeline (runs behind the tap matmuls, across
    #    blocks): stage1 = STT shortcut add + ffn1 + gelu; stage2 = ffn2 +
    #    descale + residual + output DMA. k is the global chunk index.
    pend1, pend2 = [], []

    def tail1(k, x1ps, xf, kl):
        x1b = x1pool.tile([DIM, CH], BF16, tag="x1b", name="x1b")
        nc.vector.scalar_tensor_tensor(
            x1b[:], x1ps[:], inv_s,
            xf[:, 6 + 2 * kl:8 + 2 * kl, HALO:W + HALO], OP.mult, OP.add)
        hps = ps_h.tile([DIM, 2 * CH], F32, tag="h", name="hps")
        nc.tensor.matmul(hps[:, :CH], wf1_sb[:, 0:DIM], x1b[:],
                         start=True, stop=True)
        nc.tensor.matmul(hps[:, CH:], wf1_sb[:, DIM:2 * DIM], x1b[:],
                         start=True, stop=True)
        hb = hbpool.tile([DIM, 2, CH], FP8, tag="hb", name="hb")
        nc.scalar.activation(hb[:, 0, :], hps[:, :CH], AF.Gelu, bias=b_f1a)
        nc.scalar.activation(hb[:, 1, :], hps[:, CH:], AF.Gelu, bias=b_f1b)
        return x1b, hb

    def tail2(k, x1b, hb):
        ops_ = ps_o.tile([DIM, CH], F32, tag="o", name="ops_")
        hb_ap = hb.rearrange("p k c -> p (k c)")
        part = list(hb_ap.ap[0])
        rhs_h = bass.AP(hb_ap.tensor, hb_ap.offset, [part, [CH, 2], [1, CH]])
        nc.tensor.matmul(ops_[:], wf2_sb[:], rhs_h, start=True, stop=True,
                         perf_mode=DRMODE)
        ot = opool.tile([DIM, CH], BF16, tag="ot", name="ot")
        nc.scalar.activation(ot[:], ops_[:], AF.Identity, bias=b_f2,
                             scale=inv_s2)
        out_c = opool.tile([DIM, 2, W], BF16, tag="out", name="out_c")
        nc.vector.tensor_add(out_c[:], ot[:], x1b[:])
        nc.sync.dma_start(y_d[:, 2 * k:2 * k + 2, :], out_c[:])

    def drain(n1, n2):
        while len(pend1) > n1:
            k, acc, xf, kl = pend1.pop(0)
            x1b, hb = tail1(k, acc, xf, kl)
            pend2.append((k, x1b, hb))
        while len(pend2) > n2:
            tail2(*pend2.pop(0))

    def stage_B(i, st):
        """paired-tap matmul groups for block i (+ pipelined tails)."""
        xf, xn = st["xf"], st["xn"]
        xn_flat = xn.rearrange("p r w -> p (r w)")
        part = list(xn_flat.ap[0])

        def off(row, di, dj):
            return (6 + row + di) * WP + HALO + dj

        for g in range(NXC // TAP_GROUP):
            accs = [ps_acc.tile([DIM, CH], F32, tag="acc", name="x1ps")
                    for _ in range(TAP_GROUP)]
            for p, ((diA, djA), (diB, djB)) in enumerate(PAIRS):
                step = (diB - diA) * WP + (djB - djA)
                for q in range(TAP_GROUP):
                    for h in range(2):
                        row = (TAP_GROUP * g + q) * 2 + h
                        rhs = bass.AP(xn_flat.tensor,
                                      xn_flat.offset + off(row, diA, djA),
                                      [part, [step, 2], [1, W]])
                        nc.tensor.matmul(accs[q][:, h * W:(h + 1) * W],
                                         wtp_sb[:, p, :, :], rhs,
                                         start=(p == 0 and h == 0),
                                         stop=False, perf_mode=DRMODE)
            (diS, djS) = SINGLES[0]
            for q in range(TAP_GROUP):
                for h in range(2):
                    row = (TAP_GROUP * g + q) * 2 + h
                    o = off(row, diS, djS)
                    nc.tensor.matmul(accs[q][:, h * W:(h + 1) * W],
                                     wts_sb[:], xn_flat[:, o:o + W],
                                     start=False, stop=(h == 1))
            drain(0, TAP_GROUP)
            pend1.extend(
                (NXC * i + TAP_GROUP * g + q, accs[q], xf, TAP_GROUP * g + q)
                for q in range(TAP_GROUP))

    from contextlib import ExitStack
    rep_ctx = ExitStack()
    if reps > 1:
        rep_ctx.enter_context(tc.For_i(0, reps, 1))
    st = stage_A(0)
    for i in range(NBLK):
        nxt = stage_A(i + 1) if i + 1 < NBLK else None
        stage_B(i, st)
        st = nxt
    drain(0, 0)

    rep_ctx.close()

    for p in reversed(pools):
        p.release()


# ---------------- host side ----------------

def _p2_scale(maxval, cap=224.0):
    """Largest power of two s with s*maxval <= cap."""
    if maxval <= 0:
        return 1.0
    return 2.0 ** int(np.floor(np.log2(cap / maxval)))


def prep_core(inputs, core):
    b, half = core // 2, core % 2
    x = np.asarray(inputs["x"][b], np.float32)
    xp = np.zeros((DIM, H // 2 + 2 * HALO, WP), np.float32)
    r_lo = half * (H // 2) - HALO
    s_lo, s_hi = max(0, r_lo), min(H, r_lo + H // 2 + 2 * HALO)
    xp[:, s_lo - r_lo:s_hi - r_lo, HALO:W + HALO] = x[:, s_lo:s_hi, :]

    # per-pixel LN stats planes (note: zero-padded pixels give a = 1/sqrt(eps)
    # but b = 0 and x = 0, so xn = 0 in the halo, matching conv zero-pad)
    s1 = xp.sum(axis=0)
    s2 = (xp * xp).sum(axis=0)
    var = s2 / DIM - (s1 / DIM) ** 2
    rsig = 1.0 / np.sqrt(var + EPS)
    ab = np.stack([rsig, -(s1 / DIM) * rsig]).astype(NPBF)

    w0, w1, w2 = [float(v) for v in np.asarray(inputs["scale_weights"][b],
                                               np.float64)]
    s = 1.0 + np.asarray(inputs["prompt"][b], np.float64)
    projW_s = np.asarray(inputs["proj_w"], np.float64) * s[None, :]

    e0k = np.asarray(inputs["e0_dw_w"], np.float64)[:, 0]   # [96,3,3]
    e1k = np.asarray(inputs["e1_dw_w"], np.float64)[:, 0]
    e2k = np.asarray(inputs["e2_dw_w"], np.float64)[:, 0]
    pw = np.asarray(inputs["e0_pw_w"], np.float64)          # [out, in]
    b_pw = np.asarray(inputs["e0_pw_b"], np.float64)

    # merged folded tap matrices, lhsT layout [in(xn ch), out]
    folded = {t: np.zeros((DIM, DIM)) for t in ALL_OFFS}
    for (di, dj) in TAPS_E1:
        col = w1 * e1k[:, di // 2 + 1, dj // 2 + 1]
        folded[(di, dj)] += (projW_s * col[None, :]).T
    for (di, dj) in TAPS_E2:
        col = w2 * e2k[:, di // 3 + 2, dj // 3 + 2]
        folded[(di, dj)] += (projW_s * col[None, :]).T
    for (di, dj) in TAPS_E0:
        col = w0 * e0k[:, di + 1, dj + 1]
        folded[(di, dj)] += ((projW_s * col[None, :]) @ pw).T

    s_tap = _p2_scale(max(np.abs(m).max() for m in folded.values()))
    wtp = np.zeros((NPAIR, DIM, 2, DIM), np.float64)
    for p, (tA, tB) in enumerate(PAIRS):
        wtp[p, :, 0, :] = folded[tA] * s_tap
        wtp[p, :, 1, :] = folded[tB] * s_tap
    wts = folded[SINGLES[0]] * s_tap

    f2T = np.asarray(inputs["ffn2_w"], np.float64).T    # [192, 96]
    s_f2 = _p2_scale(np.abs(f2T).max())
    wf2 = np.stack([f2T[:DIM], f2T[DIM:]], axis=1) * s_f2  # [96, 2, 96]

    # effective proj bias: dw biases through proj, plus the e0 pointwise
    # bias routed through the composed e0 taps
    cb = (w1 * np.asarray(inputs["e1_dw_b"], np.float64)
          + w2 * np.asarray(inputs["e2_dw_b"], np.float64)
          + w0 * np.asarray(inputs["e0_dw_b"], np.float64))
    e0_colsum = w0 * e0k.sum(axis=(1, 2))
    proj_b_eff = (np.asarray(inputs["proj_b"], np.float64) + projW_s @ cb
                  + projW_s @ (e0_colsum * b_pw))

    ln_b = np.asarray(inputs["ln_b"], np.float64)
    assert np.allclose(ln_b, 0.0), "kernel folds ln_b=0; got nonzero ln_b"

    # b_proj folded into ffn1 bias and the output residual constant
    f1w = np.asarray(inputs["ffn1_w"], np.float64)
    f1b_eff = np.asarray(inputs["ffn1_b"], np.float64) + f1w @ proj_b_eff
    bias = np.zeros((DIM, 8), np.float64)
    bias[:, 2] = f1b_eff[:DIM]
    bias[:, 3] = f1b_eff[DIM:]
    bias[:, 4] = np.asarray(inputs["ffn2_b"], np.float64) + proj_b_eff
    bias[:, 5] = 1.0 / s_tap
    bias[:, 7] = 1.0 / s_f2

    clip8 = lambda a: np.clip(a, -224.0, 224.0).astype(NP8)
    return {
        "xp": xp.astype(NPBF),
        "ab": ab,
        "wtp": clip8(wtp),
        "wts": clip8(wts),
        "wf1": np.asarray(inputs["ffn1_w"], np.float64).T.astype(NPBF),
        "wf2": clip8(wf2),
        "bias": bias.astype(np.float32),
    }


def kernel(**inputs):
    nc = build_nc()
    in_maps = [prep_core(inputs, c) for c in range(8)]
    res = run_bass_kernel_spmd(nc, in_maps, list(range(8)))
    out = np.empty((B, DIM, H, W), np.float32)
    for c in range(8):
        b, half = c // 2, c % 2
        out[b, :, half * (H // 2):(half + 1) * (H // 2), :] = \
            np.asarray(res.results[c]["y"], np.float32)
    return out


# revision 4
# speedup vs baseline: 1.0619x; 1.0102x over previous
"""fp8-DoubleRow Bass/Tile kernel for the MoE block (v7).

Sharding: 8 cores = 4 batches x 2 H-halves; each core computes a full
[96, 128, 256] output slab from a host-padded [96, 140, 268] bf16 slab
plus bf16 per-pixel LayerNorm scale/shift planes (host-computed,
partition-broadcast onto SBUF by DMA).

All three experts are folded into proj as 96x96 tap matmuls on xn:
  e1/e2 taps:   proj . diag((1+prompt) * w_k * dw_tap)
  e0 taps:      proj . diag(...) . pw      (pointwise conv pre-composed)
Same-offset taps merge, giving 41 unique taps -> 20 fp8 DoubleRow pairs
(two taps per matmul; rhs 3D [96, 2, 256] with even pair step — HW
constraint) + 1 single fp8 matmul per output row. ffn2 (K=192) is one
DoubleRow matmul on fp8 gelu outputs. fp8 weight scales are powers of
two, descaled via ACT scale= / DVE STT scalar.

Per-chunk tails (STT shortcut-add, ffn1, gelu, ffn2, out) run as a
two-stage software pipeline one/two tap-groups behind, across block
boundaries, so the PE never waits on freshly produced DVE/ACT data.
"""
import os
import sys

os.environ.setdefault("MYCRO_LOCAL_CACHE", "1")

import numpy as np

for _p in ("/opt/trn_rl_repo",):
    if _p not in sys.path:
        sys.path.append(_p)

import concourse.bass as bass  # noqa: E402
import concourse.bacc as bacc  # noqa: E402
import concourse.tile as tile  # noqa: E402
from concourse import mybir  # noqa: E402
from concourse.bass_utils import run_bass_kernel_spmd  # noqa: E402

F32 = mybir.dt.float32
BF16 = mybir.dt.bfloat16
FP8 = mybir.dt.float8e4
NPBF = mybir.dt.np(BF16)
NP8 = mybir.dt.np(FP8)
OP = mybir.AluOpType
AF = mybir.ActivationFunctionType
DRMODE = mybir.MatmulPerfMode.DoubleRow

DIM = 96
B, H, W = 4, 256, 256
HALO = 6
BH = 16                      # output rows per block
NBLK = (H // 2) // BH        # 8
WP = W + 2 * HALO            # 268
ROWS = BH + 2 * HALO         # 28
NPAD = ROWS * WP             # 7504
CH = 512
NCHUNK = (NPAD + CH - 1) // CH   # 15
NXC = BH * W // CH           # 8 x1/ffn chunks per block (2 rows each)
EPS = 1e-6
TAP_GROUP = 2

TAPS_E1 = [(di, dj) for di in (-2, 0, 2) for dj in (-2, 0, 2)]
TAPS_E2 = [(di, dj) for di in (-6, -3, 0, 3, 6) for dj in (-6, -3, 0, 3, 6)]
TAPS_E0 = [(di, dj) for di in (-1, 0, 1) for dj in (-1, 0, 1)]

# unique merged tap offsets, deterministic order
ALL_OFFS = sorted(set(TAPS_E1) | set(TAPS_E2) | set(TAPS_E0))
_EV = [t for t in ALL_OFFS if t[1] % 2 == 0]
_OD = [t for t in ALL_OFFS if t[1] % 2 != 0]
PAIRS = [(_EV[i], _EV[i + 1]) for i in range(0, len(_EV) - 1, 2)] + \
        [(_OD[i], _OD[i + 1]) for i in range(0, len(_OD) - 1, 2)]
SINGLES = ([_EV[-1]] if len(_EV) % 2 else []) + \
          ([_OD[-1]] if len(_OD) % 2 else [])
assert len(SINGLES) == 1 and len(PAIRS) == 20
NPAIR = len(PAIRS)

_CACHE = {}


def build_nc(reps=1):
    key = ("nc", reps)
    if key in _CACHE:
        return _CACHE[key]
    nc = bacc.Bacc("TRN2", target_bir_lowering=False, debug=False)

    xp_d = nc.dram_tensor("xp", [DIM, H // 2 + 2 * HALO, WP], BF16,
                          kind="ExternalInput")
    ab_d = nc.dram_tensor("ab", [2, H // 2 + 2 * HALO, WP], BF16,
                          kind="ExternalInput")
    wtp_d = nc.dram_tensor("wtp", [NPAIR, DIM, 2, DIM], FP8,
                           kind="ExternalInput")
    wts_d = nc.dram_tensor("wts", [DIM, DIM], FP8, kind="ExternalInput")
    wf1_d = nc.dram_tensor("wf1", [DIM, 2 * DIM], BF16, kind="ExternalInput")
    wf2_d = nc.dram_tensor("wf2", [DIM, 2, DIM], FP8, kind="ExternalInput")
    bias_d = nc.dram_tensor("bias", [DIM, 8], F32, kind="ExternalInput")
    y_d = nc.dram_tensor("y", [DIM, H // 2, W], BF16,
                         kind="ExternalOutput")

    with tile.TileContext(nc) as tc:
        _emit(nc, tc, xp_d, ab_d, wtp_d, wts_d, wf1_d, wf2_d, bias_d, y_d,
              reps)

    nc.compile()
    _CACHE[key] = nc
    return nc


def _emit(nc, tc, xp_d, ab_d, wtp_d, wts_d, wf1_d, wf2_d, bias_d, y_d,
          reps=1):
    pools = []

    wpool = tc.alloc_tile_pool(name="w", bufs=1)
    xpool = tc.alloc_tile_pool(name="xp", bufs=2)
    xnpool = tc.alloc_tile_pool(name="xn", bufs=2)
    tmppool = tc.alloc_tile_pool(name="tmp", bufs=2)
    abpool = tc.alloc_tile_pool(name="abp", bufs=2)
    x1pool = tc.alloc_tile_pool(name="x1", bufs=5)
    hbpool = tc.alloc_tile_pool(name="hb", bufs=5)
    opool = tc.alloc_tile_pool(name="o", bufs=4)
    ps_acc = tc.alloc_tile_pool(name="pacc", bufs=2 * TAP_GROUP, space="PSUM")
    ps_h = tc.alloc_tile_pool(name="ph", bufs=2, space="PSUM")
    ps_o = tc.alloc_tile_pool(name="po", bufs=2, space="PSUM")
    pools += [wpool, xpool, xnpool, tmppool, abpool, x1pool, hbpool,
              opool, ps_acc, ps_h, ps_o]

    # ---- weights / constants (loaded once) ----
    wtp_sb = wpool.tile([DIM, NPAIR, 2, DIM], FP8)
    nc.sync.dma_start(wtp_sb[:], wtp_d.ap().rearrange("t c k o -> c t k o"))
    wts_sb = wpool.tile([DIM, DIM], FP8)
    nc.sync.dma_start(wts_sb[:], wts_d[:])
    wf1_sb = wpool.tile([DIM, 2 * DIM], BF16)
    nc.sync.dma_start(wf1_sb[:], wf1_d[:])
    wf2_sb = wpool.tile([DIM, 2, DIM], FP8)
    nc.sync.dma_start(wf2_sb[:], wf2_d[:])
    bias_sb = wpool.tile([DIM, 8], F32)
    nc.sync.dma_start(bias_sb[:], bias_d[:])

    b_f1a = bias_sb[:, 2:3]
    b_f1b = bias_sb[:, 3:4]
    b_f2 = bias_sb[:, 4:5]
    inv_s = bias_sb[:, 5:6]      # 1/s for tap weights
    inv_s2 = bias_sb[:, 7:8]     # 1/s2 for ffn2

    def stage_A(i):
        """DMA (incl. partition-broadcast LN planes) + normalize block i."""
        r0 = BH * i
        xf = xpool.tile([DIM, ROWS, WP], BF16, tag="xf", name="xf")
        nc.sync.dma_start(xf[:], xp_d[:, r0:r0 + ROWS, :])
        xfF = xf.rearrange("p r w -> p (r w)")
        ab_bc = abpool.tile([DIM, 2, ROWS, WP], BF16, tag="ab", name="ab_bc")
        for pl in range(2):
            src = ab_d[pl:pl + 1, r0:r0 + ROWS, :]
            src = bass.AP(src.tensor, src.offset,
                          [[0, DIM]] + [list(q) for q in src.ap[1:]])
            nc.sync.dma_start(ab_bc[:, pl], src)
        aF = ab_bc[:, 0].rearrange("p r w -> p (r w)")
        bF = ab_bc[:, 1].rearrange("p r w -> p (r w)")
        xn = xnpool.tile([DIM, ROWS, WP], FP8, tag="xn", name="xn")
        xnF = xn.rearrange("p r w -> p (r w)")
        for j in range(NCHUNK):
            c0 = j * CH
            nj = min(CH, NPAD - c0)
            tmp = tmppool.tile([DIM, CH], BF16, tag="t", name="tmp")
            nc.vector.tensor_mul(tmp[:, :nj], xfF[:, c0:c0 + nj],
                                 aF[:, c0:c0 + nj])
            nc.gpsimd.tensor_add(xnF[:, c0:c0 + nj], tmp[:, :nj],
                                 bF[:, c0:c0 + nj])
        return dict(xf=xf, xn=xn)

    # -- two-stage tail pipeline (runs behind the tap matmuls, across
    #    blocks): stage1 = STT shortcut add + ffn1 + gelu; stage2 = ffn2 +
    #    descale + residual + output DMA. k is the global chunk index.
    pend1, pend2 = [], []

    def tail1(k, x1ps, xf, kl):
        x1b = x1pool.tile([DIM, CH], BF16, tag="x1b", name="x1b")
        nc.vector.scalar_tensor_tensor(
            x1b[:], x1ps[:], inv_s,
            xf[:, 6 + 2 * kl:8 + 2 * kl, HALO:W + HALO], OP.mult, OP.add)
        # two 1-bank psum tiles so chunk c+1's ffn1 only waits on the
        # first gelu of chunk c, not both
        hpsA = ps_h.tile([DIM, CH], F32, tag="h", name="hpsA")
        hpsB = ps_h.tile([DIM, CH], F32, tag="h", name="hpsB")
        nc.tensor.matmul(hpsA[:], wf1_sb[:, 0:DIM], x1b[:],
                         start=True, stop=True)
        nc.tensor.matmul(hpsB[:], wf1_sb[:, DIM:2 * DIM], x1b[:],
                         start=True, stop=True)
        hb = hbpool.tile([DIM, 2, CH], FP8, tag="hb", name="hb")
        nc.scalar.activation(hb[:, 0, :], hpsA[:], AF.Gelu, bias=b_f1a)
        nc.scalar.activation(hb[:, 1, :], hpsB[:], AF.Gelu, bias=b_f1b)
        return x1b, hb

    def tail2(k, x1b, hb):
        ops_ = ps_o.tile([DIM, CH], F32, tag="o", name="ops_")
        hb_ap = hb.rearrange("p k c -> p (k c)")
        part = list(hb_ap.ap[0])
        rhs_h = bass.AP(hb_ap.tensor, hb_ap.offset, [part, [CH, 2], [1, CH]])
        nc.tensor.matmul(ops_[:], wf2_sb[:], rhs_h, start=True, stop=True,
                         perf_mode=DRMODE)
        ot = opool.tile([DIM, CH], BF16, tag="ot", name="ot")
        nc.scalar.activation(ot[:], ops_[:], AF.Identity, bias=b_f2,
                             scale=inv_s2)
        out_c = opool.tile([DIM, 2, W], BF16, tag="out", name="out_c")
        nc.vector.tensor_add(out_c[:], ot[:], x1b[:])
        nc.sync.dma_start(y_d[:, 2 * k:2 * k + 2, :], out_c[:])

    def drain(n1, n2):
        while len(pend1) > n1:
            k, acc, xf, kl = pend1.pop(0)
            x1b, hb = tail1(k, acc, xf, kl)
            pend2.append((k, x1b, hb))
        while len(pend2) > n2:
            tail2(*pend2.pop(0))

    def stage_B(i, st):
        """paired-tap matmul groups for block i (+ pipelined tails)."""
        xf, xn = st["xf"], st["xn"]
        xn_flat = xn.rearrange("p r w -> p (r w)")
        part = list(xn_flat.ap[0])

        def off(row, di, dj):
            return (6 + row + di) * WP + HALO + dj

        for g in range(NXC // TAP_GROUP):
            accs = [ps_acc.tile([DIM, CH], F32, tag="acc", name="x1ps")
                    for _ in range(TAP_GROUP)]
            for p, ((diA, djA), (diB, djB)) in enumerate(PAIRS):
                step = (diB - diA) * WP + (djB - djA)
                for q in range(TAP_GROUP):
                    for h in range(2):
                        row = (TAP_GROUP * g + q) * 2 + h
                        rhs = bass.AP(xn_flat.tensor,
                                      xn_flat.offset + off(row, diA, djA),
                                      [part, [step, 2], [1, W]])
                        nc.tensor.matmul(accs[q][:, h * W:(h + 1) * W],
                                         wtp_sb[:, p, :, :], rhs,
                                         start=(p == 0 and h == 0),
                                         stop=False, perf_mode=DRMODE)
            (diS, djS) = SINGLES[0]
            for q in range(TAP_GROUP):
                for h in range(2):
                    row = (TAP_GROUP * g + q) * 2 + h
                    o = off(row, diS, djS)
                    nc.tensor.matmul(accs[q][:, h * W:(h + 1) * W],
                                     wts_sb[:], xn_flat[:, o:o + W],
                                     start=False, stop=(h == 1))
            drain(0, TAP_GROUP)
            pend1.extend(
                (NXC * i + TAP_GROUP * g + q, accs[q], xf, TAP_GROUP * g + q)
                for q in range(TAP_GROUP))

    from contextlib import ExitStack
    rep_ctx = ExitStack()
    if reps > 1:
        rep_ctx.enter_context(tc.For_i(0, reps, 1))
    st = stage_A(0)
    for i in range(NBLK):
        nxt = stage_A(i + 1) if i + 1 < NBLK else None
        stage_B(i, st)
        st = nxt
    drain(0, 0)

    rep_ctx.close()

    for p in reversed(pools):
        p.release()


# ---------------- host side ----------------

def _p2_scale(maxval, cap=224.0):
    """Largest power of two s with s*maxval <= cap."""
    if maxval <= 0:
        return 1.0
    return 2.0 ** int(np.floor(np.log2(cap / maxval)))


def prep_core(inputs, core):
    b, half = core // 2, core % 2
    x = np.asarray(inputs["x"][b], np.float32)
    xp = np.zeros((DIM, H // 2 + 2 * HALO, WP), np.float32)
    r_lo = half * (H // 2) - HALO
    s_lo, s_hi = max(0, r_lo), min(H, r_lo + H // 2 + 2 * HALO)
    xp[:, s_lo - r_lo:s_hi - r_lo, HALO:W + HALO] = x[:, s_lo:s_hi, :]

    # per-pixel LN stats planes (note: zero-padded pixels give a = 1/sqrt(eps)
    # but b = 0 and x = 0, so xn = 0 in the halo, matching conv zero-pad)
    s1 = xp.sum(axis=0)
    s2 = (xp * xp).sum(axis=0)
    var = s2 / DIM - (s1 / DIM) ** 2
    rsig = 1.0 / np.sqrt(var + EPS)
    ab = np.stack([rsig, -(s1 / DIM) * rsig]).astype(NPBF)

    w0, w1, w2 = [float(v) for v in np.asarray(inputs["scale_weights"][b],
                                               np.float64)]
    s = 1.0 + np.asarray(inputs["prompt"][b], np.float64)
    projW_s = np.asarray(inputs["proj_w"], np.float64) * s[None, :]

    e0k = np.asarray(inputs["e0_dw_w"], np.float64)[:, 0]   # [96,3,3]
    e1k = np.asarray(inputs["e1_dw_w"], np.float64)[:, 0]
    e2k = np.asarray(inputs["e2_dw_w"], np.float64)[:, 0]
    pw = np.asarray(inputs["e0_pw_w"], np.float64)          # [out, in]
    b_pw = np.asarray(inputs["e0_pw_b"], np.float64)

    # merged folded tap matrices, lhsT layout [in(xn ch), out]
    folded = {t: np.zeros((DIM, DIM)) for t in ALL_OFFS}
    for (di, dj) in TAPS_E1:
        col = w1 * e1k[:, di // 2 + 1, dj // 2 + 1]
        folded[(di, dj)] += (projW_s * col[None, :]).T
    for (di, dj) in TAPS_E2:
        col = w2 * e2k[:, di // 3 + 2, dj // 3 + 2]
        folded[(di, dj)] += (projW_s * col[None, :]).T
    for (di, dj) in TAPS_E0:
        col = w0 * e0k[:, di + 1, dj + 1]
        folded[(di, dj)] += ((projW_s * col[None, :]) @ pw).T

    s_tap = _p2_scale(max(np.abs(m).max() for m in folded.values()))
    wtp = np.zeros((NPAIR, DIM, 2, DIM), np.float64)
    for p, (tA, tB) in enumerate(PAIRS):
        wtp[p, :, 0, :] = folded[tA] * s_tap
        wtp[p, :, 1, :] = folded[tB] * s_tap
    wts = folded[SINGLES[0]] * s_tap

    f2T = np.asarray(inputs["ffn2_w"], np.float64).T    # [192, 96]
    s_f2 = _p2_scale(np.abs(f2T).max())
    wf2 = np.stack([f2T[:DIM], f2T[DIM:]], axis=1) * s_f2  # [96, 2, 96]

    # effective proj bias: dw biases through proj, plus the e0 pointwise
    # bias routed through the composed e0 taps
    cb = (w1 * np.asarray(inputs["e1_dw_b"], np.float64)
          + w2 * np.asarray(inputs["e2_dw_b"], np.float64)
          + w0 * np.asarray(inputs["e0_dw_b"], np.float64))
    e0_colsum = w0 * e0k.sum(axis=(1, 2))
    proj_b_eff = (np.asarray(inputs["proj_b"], np.float64) + projW_s @ cb
                  + projW_s @ (e0_colsum * b_pw))

    ln_b = np.asarray(inputs["ln_b"], np.float64)
    assert np.allclose(ln_b, 0.0), "kernel folds ln_b=0; got nonzero ln_b"

    # b_proj folded into ffn1 bias and the output residual constant
    f1w = np.asarray(inputs["ffn1_w"], np.float64)
    f1b_eff = np.asarray(inputs["ffn1_b"], np.float64) + f1w @ proj_b_eff
    bias = np.zeros((DIM, 8), np.float64)
    bias[:, 2] = f1b_eff[:DIM]
    bias[:, 3] = f1b_eff[DIM:]
    bias[:, 4] = np.asarray(inputs["ffn2_b"], np.float64) + proj_b_eff
    bias[:, 5] = 1.0 / s_tap
    bias[:, 7] = 1.0 / s_f2

    clip8 = lambda a: np.clip(a, -224.0, 224.0).astype(NP8)
    return {
        "xp": xp.astype(NPBF),
        "ab": ab,
        "wtp": clip8(wtp),
        "wts": clip8(wts),
        "wf1": np.asarray(inputs["ffn1_w"], np.float64).T.astype(NPBF),
        "wf2": clip8(wf2),
        "bias": bias.astype(np.float32),
    }


def kernel(**inputs):
    nc = build_nc()
    in_maps = [prep_core(inputs, c) for c in range(8)]
    res = run_bass_kernel_spmd(nc, in_maps, list(range(8)))
    out = np.empty((B, DIM, H, W), np.float32)
    for c in range(8):
        b, half = c // 2, c % 2
        out[b, :, half * (H // 2):(half + 1) * (H // 2), :] = \
            np.asarray(res.results[c]["y"], np.float32)
    return out


# revision 5
# speedup vs baseline: 1.1527x; 1.0855x over previous
"""fp8-DoubleRow Bass/Tile kernel for the MoE block (v7).

Sharding: 8 cores = 4 batches x 2 H-halves; each core computes a full
[96, 128, 256] output slab from a host-padded [96, 140, 268] bf16 slab
plus bf16 per-pixel LayerNorm scale/shift planes (host-computed,
partition-broadcast onto SBUF by DMA).

All three experts are folded into proj as 96x96 tap matmuls on xn:
  e1/e2 taps:   proj . diag((1+prompt) * w_k * dw_tap)
  e0 taps:      proj . diag(...) . pw      (pointwise conv pre-composed)
Same-offset taps merge, giving 41 unique taps -> 20 fp8 DoubleRow pairs
(two taps per matmul; rhs 3D [96, 2, 256] with even pair step — HW
constraint) + 1 single fp8 matmul per output row. ffn2 (K=192) is one
DoubleRow matmul on fp8 gelu outputs. fp8 weight scales are powers of
two, descaled via ACT scale= / DVE STT scalar.

Per-chunk tails (STT shortcut-add, ffn1, gelu, ffn2, out) run as a
two-stage software pipeline one/two tap-groups behind, across block
boundaries, so the PE never waits on freshly produced DVE/ACT data.
"""
import os
import sys

os.environ.setdefault("MYCRO_LOCAL_CACHE", "1")

import numpy as np

for _p in ("/opt/trn_rl_repo",):
    if _p not in sys.path:
        sys.path.append(_p)

import concourse.bass as bass  # noqa: E402
import concourse.bacc as bacc  # noqa: E402
import concourse.tile as tile  # noqa: E402
from concourse import mybir  # noqa: E402
from concourse.bass_utils import run_bass_kernel_spmd  # noqa: E402

F32 = mybir.dt.float32
BF16 = mybir.dt.bfloat16
FP8 = mybir.dt.float8e4
NPBF = mybir.dt.np(BF16)
NP8 = mybir.dt.np(FP8)
OP = mybir.AluOpType
AF = mybir.ActivationFunctionType
DRMODE = mybir.MatmulPerfMode.DoubleRow

DIM = 96
B, H, W = 4, 256, 256
HALO = 6
BH = 16                      # output rows per block
NBLK = (H // 2) // BH        # 8
WP = W + 2 * HALO            # 268
ROWS = BH + 2 * HALO         # 28
NPAD = ROWS * WP             # 7504
CH = 512
NCHUNK = (NPAD + CH - 1) // CH   # 15
NXC = BH * W // CH           # 8 x1/ffn chunks per block (2 rows each)
EPS = 1e-6
TAP_GROUP = 2

TAPS_E1 = [(di, dj) for di in (-2, 0, 2) for dj in (-2, 0, 2)]
TAPS_E2 = [(di, dj) for di in (-6, -3, 0, 3, 6) for dj in (-6, -3, 0, 3, 6)]
TAPS_E0 = [(di, dj) for di in (-1, 0, 1) for dj in (-1, 0, 1)]

# unique merged tap offsets, deterministic order
ALL_OFFS = sorted(set(TAPS_E1) | set(TAPS_E2) | set(TAPS_E0))
_EV = [t for t in ALL_OFFS if t[1] % 2 == 0]
_OD = [t for t in ALL_OFFS if t[1] % 2 != 0]
PAIRS = [(_EV[i], _EV[i + 1]) for i in range(0, len(_EV) - 1, 2)] + \
        [(_OD[i], _OD[i + 1]) for i in range(0, len(_OD) - 1, 2)]
SINGLES = ([_EV[-1]] if len(_EV) % 2 else []) + \
          ([_OD[-1]] if len(_OD) % 2 else [])
assert len(SINGLES) == 1 and len(PAIRS) == 20
NPAIR = len(PAIRS)

_CACHE = {}


def build_nc(reps=1):
    key = ("nc", reps)
    if key in _CACHE:
        return _CACHE[key]
    nc = bacc.Bacc("TRN2", target_bir_lowering=False, debug=False)

    xp_d = nc.dram_tensor("xp", [DIM, H // 2 + 2 * HALO, WP], BF16,
                          kind="ExternalInput")
    ab_d = nc.dram_tensor("ab", [2, H // 2 + 2 * HALO, WP], BF16,
                          kind="ExternalInput")
    wtp_d = nc.dram_tensor("wtp", [NPAIR, DIM, 2, DIM], FP8,
                           kind="ExternalInput")
    wts_d = nc.dram_tensor("wts", [DIM, DIM], FP8, kind="ExternalInput")
    wf1_d = nc.dram_tensor("wf1", [DIM, 2 * DIM], BF16, kind="ExternalInput")
    wf2_d = nc.dram_tensor("wf2", [DIM, 2, DIM], FP8, kind="ExternalInput")
    bias_d = nc.dram_tensor("bias", [DIM, 8], F32, kind="ExternalInput")
    y_d = nc.dram_tensor("y", [DIM, H // 2, W], BF16,
                         kind="ExternalOutput")

    with tile.TileContext(nc) as tc:
        _emit(nc, tc, xp_d, ab_d, wtp_d, wts_d, wf1_d, wf2_d, bias_d, y_d,
              reps)

    nc.compile()
    _CACHE[key] = nc
    return nc


def _emit(nc, tc, xp_d, ab_d, wtp_d, wts_d, wf1_d, wf2_d, bias_d, y_d,
          reps=1):
    pools = []

    wpool = tc.alloc_tile_pool(name="w", bufs=1)
    xpool = tc.alloc_tile_pool(name="xp", bufs=2)
    xnpool = tc.alloc_tile_pool(name="xn", bufs=2)
    tmppool = tc.alloc_tile_pool(name="tmp", bufs=2)
    abpool = tc.alloc_tile_pool(name="abp", bufs=2)
    x1pool = tc.alloc_tile_pool(name="x1", bufs=5)
    hbpool = tc.alloc_tile_pool(name="hb", bufs=5)
    opool = tc.alloc_tile_pool(name="o", bufs=4)
    ps_acc = tc.alloc_tile_pool(name="pacc", bufs=2 * TAP_GROUP, space="PSUM")
    ps_h = tc.alloc_tile_pool(name="ph", bufs=2, space="PSUM")
    ps_o = tc.alloc_tile_pool(name="po", bufs=2, space="PSUM")
    pools += [wpool, xpool, xnpool, tmppool, abpool, x1pool, hbpool,
              opool, ps_acc, ps_h, ps_o]

    # ---- weights / constants (loaded once) ----
    wtp_sb = wpool.tile([DIM, NPAIR, 2, DIM], FP8)
    nc.sync.dma_start(wtp_sb[:], wtp_d.ap().rearrange("t c k o -> c t k o"))
    wts_sb = wpool.tile([DIM, DIM], FP8)
    nc.sync.dma_start(wts_sb[:], wts_d[:])
    wf1_sb = wpool.tile([DIM, 2 * DIM], BF16)
    nc.sync.dma_start(wf1_sb[:], wf1_d[:])
    wf2_sb = wpool.tile([DIM, 2, DIM], FP8)
    nc.sync.dma_start(wf2_sb[:], wf2_d[:])
    bias_sb = wpool.tile([DIM, 8], F32)
    nc.sync.dma_start(bias_sb[:], bias_d[:])

    b_f1a = bias_sb[:, 2:3]
    b_f1b = bias_sb[:, 3:4]
    b_f2 = bias_sb[:, 4:5]
    inv_s = bias_sb[:, 5:6]      # 1/s for tap weights
    inv_s2 = bias_sb[:, 7:8]     # 1/s2 for ffn2

    def stage_A_dma(i):
        """Issue DMAs (incl. partition-broadcast LN planes) for block i."""
        r0 = BH * i
        xf = xpool.tile([DIM, ROWS, WP], BF16, tag="xf", name="xf")
        nc.sync.dma_start(xf[:], xp_d[:, r0:r0 + ROWS, :])
        ab_bc = abpool.tile([DIM, 2, ROWS, WP], BF16, tag="ab", name="ab_bc")
        for pl in range(2):
            src = ab_d[pl:pl + 1, r0:r0 + ROWS, :]
            src = bass.AP(src.tensor, src.offset,
                          [[0, DIM]] + [list(q) for q in src.ap[1:]])
            nc.sync.dma_start(ab_bc[:, pl], src)
        xn = xnpool.tile([DIM, ROWS, WP], FP8, tag="xn", name="xn")
        return dict(xf=xf, ab=ab_bc, xn=xn)

    def stage_A_compute(st, j0, j1):
        """Normalize chunks [j0, j1) of a block; emitted in slices between
        tap groups so these DVE muls never queue ahead of urgent tail
        STTs (in-order DVE queue caused ~6us PE stalls per block)."""
        xfF = st["xf"].rearrange("p r w -> p (r w)")
        aF = st["ab"][:, 0].rearrange("p r w -> p (r w)")
        bF = st["ab"][:, 1].rearrange("p r w -> p (r w)")
        xnF = st["xn"].rearrange("p r w -> p (r w)")
        for j in range(j0, min(j1, NCHUNK)):
            c0 = j * CH
            nj = min(CH, NPAD - c0)
            tmp = tmppool.tile([DIM, CH], BF16, tag="t", name="tmp")
            nc.vector.tensor_mul(tmp[:, :nj], xfF[:, c0:c0 + nj],
                                 aF[:, c0:c0 + nj])
            nc.gpsimd.tensor_add(xnF[:, c0:c0 + nj], tmp[:, :nj],
                                 bF[:, c0:c0 + nj])

    # -- two-stage tail pipeline (runs behind the tap matmuls, across
    #    blocks): stage1 = STT shortcut add + ffn1 + gelu; stage2 = ffn2 +
    #    descale + residual + output DMA. k is the global chunk index.
    pend1, pend2 = [], []

    def tail1(k, x1ps, xf, kl):
        x1b = x1pool.tile([DIM, CH], BF16, tag="x1b", name="x1b")
        nc.vector.scalar_tensor_tensor(
            x1b[:], x1ps[:], inv_s,
            xf[:, 6 + 2 * kl:8 + 2 * kl, HALO:W + HALO], OP.mult, OP.add)
        # two 1-bank psum tiles so chunk c+1's ffn1 only waits on the
        # first gelu of chunk c, not both
        hpsA = ps_h.tile([DIM, CH], F32, tag="h", name="hpsA")
        hpsB = ps_h.tile([DIM, CH], F32, tag="h", name="hpsB")
        nc.tensor.matmul(hpsA[:], wf1_sb[:, 0:DIM], x1b[:],
                         start=True, stop=True)
        nc.tensor.matmul(hpsB[:], wf1_sb[:, DIM:2 * DIM], x1b[:],
                         start=True, stop=True)
        hb = hbpool.tile([DIM, 2, CH], FP8, tag="hb", name="hb")
        nc.scalar.activation(hb[:, 0, :], hpsA[:], AF.Gelu, bias=b_f1a)
        nc.scalar.activation(hb[:, 1, :], hpsB[:], AF.Gelu, bias=b_f1b)
        return x1b, hb

    def tail2(k, x1b, hb):
        ops_ = ps_o.tile([DIM, CH], F32, tag="o", name="ops_")
        hb_ap = hb.rearrange("p k c -> p (k c)")
        part = list(hb_ap.ap[0])
        rhs_h = bass.AP(hb_ap.tensor, hb_ap.offset, [part, [CH, 2], [1, CH]])
        nc.tensor.matmul(ops_[:], wf2_sb[:], rhs_h, start=True, stop=True,
                         perf_mode=DRMODE)
        ot = opool.tile([DIM, CH], BF16, tag="ot", name="ot")
        nc.scalar.activation(ot[:], ops_[:], AF.Identity, bias=b_f2,
                             scale=inv_s2)
        out_c = opool.tile([DIM, 2, W], BF16, tag="out", name="out_c")
        nc.vector.tensor_add(out_c[:], ot[:], x1b[:])
        nc.sync.dma_start(y_d[:, 2 * k:2 * k + 2, :], out_c[:])

    def drain(n1, n2):
        while len(pend1) > n1:
            k, acc, xf, kl = pend1.pop(0)
            x1b, hb = tail1(k, acc, xf, kl)
            pend2.append((k, x1b, hb))
        while len(pend2) > n2:
            tail2(*pend2.pop(0))

    def stage_B(i, st, nxt):
        """paired-tap matmul groups for block i (+ pipelined tails +
        interleaved normalize slices for block i+1)."""
        xf, xn = st["xf"], st["xn"]
        xn_flat = xn.rearrange("p r w -> p (r w)")
        part = list(xn_flat.ap[0])

        def off(row, di, dj):
            return (6 + row + di) * WP + HALO + dj

        for g in range(NXC // TAP_GROUP):
            accs = [ps_acc.tile([DIM, CH], F32, tag="acc", name="x1ps")
                    for _ in range(TAP_GROUP)]
            for p, ((diA, djA), (diB, djB)) in enumerate(PAIRS):
                step = (diB - diA) * WP + (djB - djA)
                for q in range(TAP_GROUP):
                    for h in range(2):
                        row = (TAP_GROUP * g + q) * 2 + h
                        rhs = bass.AP(xn_flat.tensor,
                                      xn_flat.offset + off(row, diA, djA),
                                      [part, [step, 2], [1, W]])
                        nc.tensor.matmul(accs[q][:, h * W:(h + 1) * W],
                                         wtp_sb[:, p, :, :], rhs,
                                         start=(p == 0 and h == 0),
                                         stop=False, perf_mode=DRMODE)
            (diS, djS) = SINGLES[0]
            for q in range(TAP_GROUP):
                for h in range(2):
                    row = (TAP_GROUP * g + q) * 2 + h
                    o = off(row, diS, djS)
                    nc.tensor.matmul(accs[q][:, h * W:(h + 1) * W],
                                     wts_sb[:], xn_flat[:, o:o + W],
                                     start=False, stop=(h == 1))
            drain(0, TAP_GROUP)
            pend1.extend(
                (NXC * i + TAP_GROUP * g + q, accs[q], xf, TAP_GROUP * g + q)
                for q in range(TAP_GROUP))
            if nxt is not None:
                per = -(-NCHUNK // (NXC // TAP_GROUP))
                stage_A_compute(nxt, per * g, per * (g + 1))

    from contextlib import ExitStack
    rep_ctx = ExitStack()
    if reps > 1:
        rep_ctx.enter_context(tc.For_i(0, reps, 1))
    st = stage_A_dma(0)
    stage_A_compute(st, 0, NCHUNK)
    for i in range(NBLK):
        nxt = stage_A_dma(i + 1) if i + 1 < NBLK else None
        stage_B(i, st, nxt)
        st = nxt
    drain(0, 0)

    rep_ctx.close()

    for p in reversed(pools):
        p.release()


# ---------------- host side ----------------

def _p2_scale(maxval, cap=224.0):
    """Largest power of two s with s*maxval <= cap."""
    if maxval <= 0:
        return 1.0
    return 2.0 ** int(np.floor(np.log2(cap / maxval)))


def prep_core(inputs, core):
    b, half = core // 2, core % 2
    x = np.asarray(inputs["x"][b], np.float32)
    xp = np.zeros((DIM, H // 2 + 2 * HALO, WP), np.float32)
    r_lo = half * (H // 2) - HALO
    s_lo, s_hi = max(0, r_lo), min(H, r_lo + H // 2 + 2 * HALO)
    xp[:, s_lo - r_lo:s_hi - r_lo, HALO:W + HALO] = x[:, s_lo:s_hi, :]

    # per-pixel LN stats planes (note: zero-padded pixels give a = 1/sqrt(eps)
    # but b = 0 and x = 0, so xn = 0 in the halo, matching conv zero-pad)
    s1 = xp.sum(axis=0)
    s2 = (xp * xp).sum(axis=0)
    var = s2 / DIM - (s1 / DIM) ** 2
    rsig = 1.0 / np.sqrt(var + EPS)
    ab = np.stack([rsig, -(s1 / DIM) * rsig]).astype(NPBF)

    w0, w1, w2 = [float(v) for v in np.asarray(inputs["scale_weights"][b],
                                               np.float64)]
    s = 1.0 + np.asarray(inputs["prompt"][b], np.float64)
    projW_s = np.asarray(inputs["proj_w"], np.float64) * s[None, :]

    e0k = np.asarray(inputs["e0_dw_w"], np.float64)[:, 0]   # [96,3,3]
    e1k = np.asarray(inputs["e1_dw_w"], np.float64)[:, 0]
    e2k = np.asarray(inputs["e2_dw_w"], np.float64)[:, 0]
    pw = np.asarray(inputs["e0_pw_w"], np.float64)          # [out, in]
    b_pw = np.asarray(inputs["e0_pw_b"], np.float64)

    # merged folded tap matrices, lhsT layout [in(xn ch), out]
    folded = {t: np.zeros((DIM, DIM)) for t in ALL_OFFS}
    for (di, dj) in TAPS_E1:
        col = w1 * e1k[:, di // 2 + 1, dj // 2 + 1]
        folded[(di, dj)] += (projW_s * col[None, :]).T
    for (di, dj) in TAPS_E2:
        col = w2 * e2k[:, di // 3 + 2, dj // 3 + 2]
        folded[(di, dj)] += (projW_s * col[None, :]).T
    for (di, dj) in TAPS_E0:
        col = w0 * e0k[:, di + 1, dj + 1]
        folded[(di, dj)] += ((projW_s * col[None, :]) @ pw).T

    s_tap = _p2_scale(max(np.abs(m).max() for m in folded.values()))
    wtp = np.zeros((NPAIR, DIM, 2, DIM), np.float64)
    for p, (tA, tB) in enumerate(PAIRS):
        wtp[p, :, 0, :] = folded[tA] * s_tap
        wtp[p, :, 1, :] = folded[tB] * s_tap
    wts = folded[SINGLES[0]] * s_tap

    f2T = np.asarray(inputs["ffn2_w"], np.float64).T    # [192, 96]
    s_f2 = _p2_scale(np.abs(f2T).max())
    wf2 = np.stack([f2T[:DIM], f2T[DIM:]], axis=1) * s_f2  # [96, 2, 96]

    # effective proj bias: dw biases through proj, plus the e0 pointwise
    # bias routed through the composed e0 taps
    cb = (w1 * np.asarray(inputs["e1_dw_b"], np.float64)
          + w2 * np.asarray(inputs["e2_dw_b"], np.float64)
          + w0 * np.asarray(inputs["e0_dw_b"], np.float64))
    e0_colsum = w0 * e0k.sum(axis=(1, 2))
    proj_b_eff = (np.asarray(inputs["proj_b"], np.float64) + projW_s @ cb
                  + projW_s @ (e0_colsum * b_pw))

    ln_b = np.asarray(inputs["ln_b"], np.float64)
    assert np.allclose(ln_b, 0.0), "kernel folds ln_b=0; got nonzero ln_b"

    # b_proj folded into ffn1 bias and the output residual constant
    f1w = np.asarray(inputs["ffn1_w"], np.float64)
    f1b_eff = np.asarray(inputs["ffn1_b"], np.float64) + f1w @ proj_b_eff
    bias = np.zeros((DIM, 8), np.float64)
    bias[:, 2] = f1b_eff[:DIM]
    bias[:, 3] = f1b_eff[DIM:]
    bias[:, 4] = np.asarray(inputs["ffn2_b"], np.float64) + proj_b_eff
    bias[:, 5] = 1.0 / s_tap
    bias[:, 7] = 1.0 / s_f2

    clip8 = lambda a: np.clip(a, -224.0, 224.0).astype(NP8)
    return {
        "xp": xp.astype(NPBF),
        "ab": ab,
        "wtp": clip8(wtp),
        "wts": clip8(wts),
        "wf1": np.asarray(inputs["ffn1_w"], np.float64).T.astype(NPBF),
        "wf2": clip8(wf2),
        "bias": bias.astype(np.float32),
    }


def kernel(**inputs):
    nc = build_nc()
    in_maps = [prep_core(inputs, c) for c in range(8)]
    res = run_bass_kernel_spmd(nc, in_maps, list(range(8)))
    out = np.empty((B, DIM, H, W), np.float32)
    for c in range(8):
        b, half = c // 2, c % 2
        out[b, :, half * (H // 2):(half + 1) * (H // 2), :] = \
            np.asarray(res.results[c]["y"], np.float32)
    return out
